# revision 17
# baseline (speedup 1.0000x reference)
"""AttentionBlock Trainium2 kernel (v3: fp8 DoubleRow core + pipelined emission).

Problem: B=16, C=256, H=W=32 (N=1024 pixels), GroupNorm(8) -> 1x1-conv QKV ->
softmax attention over pixels -> 1x1-conv proj -> +residual.

Sharding: data-parallel over batch across 8 NeuronCores (2 batch elems/core),
weights replicated.  Weights are pre-transposed on the host so every matmul
operand DMAs contiguously.

Device layout per batch element (channels on partitions):
  x                  : [C=256, N=1024] fp32 (residual + groupnorm stats)
  xn                 : [C, N] bf16 (full-rate PE matmuls, fast LDWEIGHTS)
  q, k               : [C, N] fp8e4 -> scores via one DoubleRow matmul per
                       (mb, nf): contraction C=256 packed 2/partition at
                       the fp8 2x FLOP rate
  v_aug              : [N, C+8] fp8e4 (pixels on partitions; cols 256.. = ones
                       so the P@V matmul also produces the softmax row-sums)
  expS = exp(S/16-c) : [m, n] fp8e4, exp'd by ScalarE straight out of PSUM
                       (constant logit shift cancels in the normalize;
                       max logit ~5.6 -> exp in fp8e4 range, max 240)
  att^T = P@V        : 4 DoubleRow matmuls per n-block (m packed in pairs),
                       then PE-transposed to [C, n] bf16 for the proj.
Residual + proj bias are fused into the PSUM->SBUF move with one
scalar_tensor_tensor op, keeping the residual exact fp32.

Scheduling: the PE instruction stream is interleaved across phases so it
never stalls on ACT exp drains -- batch 0's score chunks alternate with
batch 1's QKV groups, and batch 1's score chunks alternate with batch 0's
PV/transpose groups.  Both activation tables (Sqrt, Exp) are preloaded by
dummy ops while the input DMAs are in flight; batch 0's QKV drains run on
the otherwise-idle ACT engine, batch 1's on DVE.
"""

from contextlib import ExitStack

import numpy as np

import concourse.bass as bass
import concourse.tile as tile
from concourse import bacc, mybir
from concourse._compat import with_exitstack
from concourse.bass_utils import run_bass_kernel_spmd

# Problem constants (hardcoded per contract)
B, C, H, W = 16, 256, 32, 32
N = H * W            # 1024 pixels
G = 8                # groups
GS = C // G          # 32 channels / group
NCORES = 8
BPC = B // NCORES    # batch elems per core
EPS = 1e-5
P = 128              # partitions
KT = C // P          # 2 c-tiles
NB = N // P          # 8 pixel blocks of 128
NF = N // 512        # 2 free chunks of 512
EXP_SHIFT = -1.5     # constant logit shift; cancels in softmax normalize
F32 = mybir.dt.float32
BF16 = mybir.dt.bfloat16
FP8 = mybir.dt.float8e4
AF = mybir.ActivationFunctionType
OP = mybir.AluOpType
DR = mybir.MatmulPerfMode.DoubleRow


@with_exitstack
def attn_tile_kernel(
    ctx: ExitStack,
    tc: tile.TileContext,
    out_d,
    x_d,
    gamma_d,
    beta_d,
    wqkvT_d,
    bqkv_d,
    wprojT_d,
    bproj_d,
    ident_d,
    gmask_d,
    gmask2_d,
    has_qkv_bias: bool,
    has_proj_bias: bool,
):
    nc = tc.nc

    consts = ctx.enter_context(tc.tile_pool(name="consts", bufs=1))
    gn = ctx.enter_context(tc.tile_pool(name="gn", bufs=2))
    big = ctx.enter_context(tc.tile_pool(name="big", bufs=2))
    es = ctx.enter_context(tc.tile_pool(name="es", bufs=2))
    # PSUM budget (8 banks), split by draining engine so phases don't
    # block each other on slots:
    #   mm  = 5x [128,512] qkv/scores/proj psums (DVE/ACT drains) (5 banks)
    #   pv  = 3x [128,264] v/PV accum + transposes + gn stats     (3 banks)
    psmm = ctx.enter_context(tc.tile_pool(name="psmm", bufs=5, space="PSUM"))
    psqp = psmm
    pspv = ctx.enter_context(tc.tile_pool(name="pspv", bufs=3, space="PSUM"))
    pssm = pspv

    state = {}

    # batch inputs go first: they gate the groupnorm -> QKV critical path,
    # while weights aren't needed until the first matmul ~8us later.
    # x0 in fine chunks (bn_stats starts per chunk), x1 whole-tile on the
    # gpsimd queue (latency uncritical, keeps the sync queue short).
    xt0 = big.tile([P, KT, N], BF16, tag="xt")
    for kt in range(KT):
        for sub in range(2):
            nc.sync.dma_start(
                xt0[:, kt, sub * 512:(sub + 1) * 512],
                x_d[0, kt * P:(kt + 1) * P, sub * 512:(sub + 1) * 512])
    state[0] = {"xt": xt0}
    xt1 = big.tile([P, KT, N], BF16, tag="xt")
    for kt in range(KT):
        nc.sync.dma_start(xt1[:, kt, :], x_d[1, kt * P:(kt + 1) * P, :])
    state[1] = {"xt": xt1}

    # ---- constants / weights (loaded once, replicated across cores) ----
    # DMA issue cost is ~0.7-0.9us per descriptor per engine queue, so the
    # issues are spread across otherwise-idle engine queues: x0 owns the
    # sync queue (it gates groupnorm), x1 + stores go via gpsimd, small
    # groupnorm consts via the tensor queue, weights via the scalar queue.
    gmask_t = consts.tile([P, 4], F32)
    nc.gpsimd.dma_start(gmask_t[:], gmask_d.ap())
    gmask2_t = consts.tile([4, P], F32)
    nc.gpsimd.dma_start(gmask2_t[:], gmask2_d.ap())
    gamma_t = consts.tile([P, KT], F32)
    beta_t = consts.tile([P, KT], F32)
    for kt in range(KT):
        nc.gpsimd.dma_start(gamma_t[:, kt:kt + 1], gamma_d[kt * P:(kt + 1) * P, :])
        nc.gpsimd.dma_start(beta_t[:, kt:kt + 1], beta_d[kt * P:(kt + 1) * P, :])
    eps_t = consts.tile([P, 1], F32)
    nc.vector.memset(eps_t, EPS)
    # warm-up matmuls on junk data: the PE clock ramps to full speed only
    # after ~3us of continuous work, so spin it up while DMAs are in flight
    warm_sb = consts.tile([P, 512], BF16)
    nc.vector.memset(warm_sb[:], 0.0)
    for _ in range(16):
        warm_ps = psmm.tile([P, 512], F32, tag="mm", name="warm_ps")
        nc.tensor.matmul(warm_ps[:], warm_sb[:, 0:P], warm_sb[:],
                         start=True, stop=True)
    shift_t = consts.tile([P, 1], F32)
    nc.vector.memset(shift_t, EXP_SHIFT)
    # preload both ACT tables (Sqrt for groupnorm, Exp for softmax) while
    # the input DMAs are still in flight so neither load sits on the
    # critical path later
    warm_t = consts.tile([P, 1], F32)
    nc.scalar.activation(warm_t[:], eps_t[:], AF.Sqrt, bias=eps_t[:], scale=1.0)
    nc.scalar.activation(warm_t[:], eps_t[:], AF.Exp, bias=shift_t[:], scale=1.0)
    # weights arrive pre-cast to bf16 from the host and go on the sync
    # queue AFTER x0/x1: DMA transfers share bandwidth, so every byte saved
    # or deferred here pulls the groupnorm -> QKV critical path earlier
    wqkv_t = consts.tile([P, KT, 3 * C], BF16)
    for kt in range(KT):
        nc.sync.dma_start(wqkv_t[:, kt, :], wqkvT_d[kt * P:(kt + 1) * P, :])
    wproj_b = consts.tile([P, KT, C], BF16)
    for kt in range(KT):
        nc.sync.dma_start(wproj_b[:, kt, :], wprojT_d[kt * P:(kt + 1) * P, :])
    ident_t = consts.tile([P, P], BF16)
    nc.sync.dma_start(ident_t[:], ident_d.ap())
    # q/k biases as per-partition columns (q: rows 0..255, k: 256..511)
    if has_qkv_bias:
        bqk_t = consts.tile([P, 2 * KT], F32)
        for j in range(2 * KT):
            nc.gpsimd.dma_start(bqk_t[:, j:j + 1], bqkv_d[j * P:(j + 1) * P, :])
        # v bias lives on the free axis of v_aug -> needs a row layout + ones col
        bv_row = consts.tile([1, C], F32)
        nc.gpsimd.dma_start(
            bv_row[:], bqkv_d[2 * C:3 * C, :].rearrange("c one -> one c"))
        ones_row_f32 = consts.tile([1, P], F32)
        nc.vector.memset(ones_row_f32, 1.0)
    if has_proj_bias:
        bproj_t = consts.tile([P, KT], F32)
        for kt in range(KT):
            nc.gpsimd.dma_start(bproj_t[:, kt:kt + 1], bproj_d[kt * P:(kt + 1) * P, :])

    def phase_load_gn(b):
        # ---- group norm stats -> per-channel scale/bias -> xn (bf16) ----
        # both c-tiles flow through one PE reduce + one [P,2]-wide chain
        st = state[b]
        xt = st["xt"]
        stats = gn.tile([P, KT, 2, 6], F32, tag="stats")
        mv = gn.tile([P, KT, 2], F32, tag="mv")
        gst = gn.tile([P, 2, KT], F32, tag="gst")  # stat-major: means | m2s
        for kt in range(KT):
            for sub in range(2):
                nc.vector.bn_stats(stats[:, kt, sub, :], xt[:, kt, sub * 512:(sub + 1) * 512])
            nc.vector.bn_aggr(mv[:, kt, :], stats[:, kt, :, :])
            nc.vector.tensor_copy(gst[:, 0, kt:kt + 1], mv[:, kt, 0:1])
            nc.vector.tensor_scalar(
                out=gst[:, 1, kt:kt + 1], in0=mv[:, kt, 0:1],
                scalar1=mv[:, kt, 0:1], scalar2=mv[:, kt, 1:2],
                op0=OP.mult, op1=OP.add,
            )
        gsum = pssm.tile([4, 2 * KT], F32, tag="pv")
        nc.tensor.matmul(gsum[:], gmask_t[:], gst[:, :, :], start=True, stop=True)
        gsb = gn.tile([4, 2 * KT], F32, tag="gsb")
        nc.vector.tensor_copy(gsb[:], gsum[:])
        gbc = pssm.tile([P, 2 * KT], F32, tag="pv")
        nc.tensor.matmul(gbc[:], gmask2_t[:], gsb[:], start=True, stop=True)
        gch = gn.tile([P, 2, KT], F32, tag="gch")  # means | m2s per channel
        nc.vector.tensor_copy(gch[:], gbc[:])
        m2 = gn.tile([P, KT], F32, tag="m2")
        nc.vector.tensor_mul(m2[:], gch[:, 0, :], gch[:, 0, :])
        varg = gn.tile([P, KT], F32, tag="varg")
        nc.vector.tensor_sub(varg[:], gch[:, 1, :], m2[:])
        sd = gn.tile([P, KT], F32, tag="sd")
        nc.scalar.activation(sd[:], varg[:], AF.Sqrt, bias=eps_t[:], scale=1.0)
        rstd = gn.tile([P, KT], F32, tag="rstd")
        nc.vector.reciprocal(rstd[:], sd[:])
        scale_c = gn.tile([P, KT], F32, tag="scale_c")
        nc.vector.tensor_mul(scale_c[:], rstd[:], gamma_t[:])
        tmp = gn.tile([P, KT], F32, tag="tmp")
        nc.vector.tensor_mul(tmp[:], gch[:, 0, :], scale_c[:])
        bias_c = gn.tile([P, KT], F32, tag="bias_c")
        nc.vector.tensor_sub(bias_c[:], beta_t[:], tmp[:])
        xn = big.tile([P, KT, N], BF16, tag="xn")
        for kt in range(KT):
            nc.vector.tensor_scalar(
                out=xn[:, kt, :], in0=xt[:, kt, :],
                scalar1=scale_c[:, kt:kt + 1], scalar2=bias_c[:, kt:kt + 1],
                op0=OP.mult, op1=OP.add,
            )
        st["xn"] = xn

    def qkv_alloc(b):
        st = state[b]
        st["q"] = big.tile([P, KT, N], FP8, tag=f"q{b}", name=f"q{b}")
        st["k"] = big.tile([P, KT, N], FP8, tag=f"k{b}", name=f"k{b}")
        st["v"] = big.tile([P, NB, C + 8], FP8, tag=f"v{b}", name=f"v{b}")

    def qk_unit(b, which, m, nf, drain_act=False):
        # one [128, 512] chunk of q (which=0) or k (which=1); single psum,
        # kt accumulated in place so the chunk drains as soon as possible
        st = state[b]
        xn = st["xn"]
        dst = st["q"] if which == 0 else st["k"]
        ocol = which * C + m * P
        ps = psqp.tile([P, 512], F32, tag="mm")
        for kt in range(KT):
            nc.tensor.matmul(
                ps[:],
                wqkv_t[:, kt, ocol:ocol + P],
                xn[:, kt, nf * 512:(nf + 1) * 512],
                start=(kt == 0), stop=(kt == KT - 1),
            )
        dstap = dst[:, m, nf * 512:(nf + 1) * 512]
        if has_qkv_bias:
            nc.vector.tensor_scalar_add(
                dstap, ps[:], bqk_t[:, 2 * which + m:2 * which + m + 1])
        elif drain_act:
            nc.scalar.copy(dstap, ps[:])
        else:
            nc.vector.tensor_copy(dstap, ps[:])

    def v_group(b, nb):
        # one pixel-block of v^T (pixels on partitions) + ones columns
        st = state[b]
        xn, v_aug = st["xn"], st["v"]
        ps = pspv.tile([P, C + 8], F32, tag="pv")
        for kt in range(KT):
            nc.tensor.matmul(
                ps[:, 0:C],
                xn[:, kt, nb * P:(nb + 1) * P],
                wqkv_t[:, kt, 2 * C:3 * C],
                start=(kt == 0), stop=(kt == KT - 1 and not has_qkv_bias),
            )
        if has_qkv_bias:
            nc.tensor.matmul(ps[:, 0:C], ones_row_f32[:], bv_row[:],
                             start=False, stop=True)
        nc.vector.tensor_copy(v_aug[:, nb, 0:C], ps[:, 0:C])
        nc.vector.memset(v_aug[:, nb, C:C + 8], 1.0)

    def scores_alloc(b):
        state[b]["expS"] = es.tile([P, NB, N], FP8, tag=f"expS{b}",
                                   name=f"expS{b}")

    def scores_group(b, mb):
        # S^T = k^T q for one m-block: a single DoubleRow matmul per nf
        # (both c-tiles contracted in one pass), exp'd chunkwise by ACT
        st = state[b]
        q_sb, k_sb, expS = st["q"], st["k"], st["expS"]
        for nf in range(NF):
            ps = psmm.tile([P, 512], F32, tag="mm")
            nc.tensor.matmul(
                ps[:],
                k_sb[:, :, mb * P:(mb + 1) * P],
                q_sb[:, :, nf * 512:(nf + 1) * 512],
                start=True, stop=True, perf_mode=DR,
            )
            nc.scalar.activation(expS[:, mb, nf * 512:(nf + 1) * 512], ps[:],
                                 AF.Exp, bias=shift_t[:], scale=1.0 / 16.0)

    def pv_group(b, nb):
        # one n-block of att^T = softmax(S) @ V (col C holds row-sums);
        # m contracted in 4 DoubleRow pairs
        st = state[b]
        expS, v_aug, attT = st["expS"], st["v"], st["attT"]
        ps = pspv.tile([P, C + 8], F32, tag="pv")
        for j in range(NB // 2):
            nc.tensor.matmul(
                ps[:, 0:C + 8],
                expS[:, 2 * j:2 * j + 2, nb * P:(nb + 1) * P],
                v_aug[:, 2 * j:2 * j + 2, 0:C + 8],
                start=(j == 0), stop=(j == NB // 2 - 1), perf_mode=DR,
            )
        rinv = gn.tile([P, 1], F32, tag="rinv")
        nc.vector.reciprocal(rinv[:], ps[:, C:C + 1])
        if b == 0:
            # batch 0 normalizes on DVE (ACT may still be draining exps)
            nc.vector.tensor_scalar_mul(attT[:, nb, :], ps[:, 0:C], rinv[:])
        else:
            # batch 1 normalizes on ACT's free affine (idle post-softmax):
            # out = Copy(in * scale), scale = per-partition rinv
            nc.scalar.activation(attT[:, nb, :], ps[:, 0:C], AF.Copy,
                                 bias=0.0, scale=rinv[:])

    def phase_pv_alloc(b):
        st = state[b]
        st["attT"] = big.tile([P, NB, C], BF16, tag=f"attT{b}", name=f"attT{b}")
        st["att"] = big.tile([P, KT, N], BF16, tag=f"att{b}", name=f"att{b}")
        st["out_sb"] = big.tile([P, KT, N], F32, tag=f"outsb{b}",
                                name=f"outsb{b}")

    def transpose_group(b, nb):
        # transpose one att^T block -> att [*, n-block] via PE (both c-blocks)
        st = state[b]
        attT, att = st["attT"], st["att"]
        for cb in range(KT):
            pt = pssm.tile([P, P], BF16, tag="pv")
            nc.tensor.transpose(pt[:], attT[:, nb, cb * P:(cb + 1) * P], ident_t[:])
            nc.vector.tensor_copy(att[:, cb, nb * P:(nb + 1) * P], pt[:])

    def proj_unit(b, cb, nf):
        # one [128, 512] proj chunk + fused (+bias)+residual on the
        # PSUM->SBUF move, then store (DMA issues spread across idle queues)
        st = state[b]
        att, xt, out_sb = st["att"], st["xt"], st["out_sb"]
        ps = psqp.tile([P, 512], F32, tag="mm")
        for kt in range(KT):
            nc.tensor.matmul(
                ps[:],
                wproj_b[:, kt, cb * P:(cb + 1) * P],
                att[:, kt, nf * 512:(nf + 1) * 512],
                start=(kt == 0), stop=(kt == KT - 1),
            )
        bias_arg = bproj_t[:, cb:cb + 1] if has_proj_bias else 0.0
        nc.vector.scalar_tensor_tensor(
            out=out_sb[:, cb, nf * 512:(nf + 1) * 512],
            in0=ps[:], scalar=bias_arg,
            in1=xt[:, cb, nf * 512:(nf + 1) * 512],
            op0=OP.add, op1=OP.add,
        )
        dma_eng = (nc.gpsimd if b == 0 else (nc.sync if nf == 0 else nc.scalar))
        dma_eng.dma_start(
            out_d[b, cb * P:(cb + 1) * P, nf * 512:(nf + 1) * 512],
            out_sb[:, cb, nf * 512:(nf + 1) * 512])

    # ---- software-pipelined emission ------------------------------------
    # Engines run their per-engine streams in order, so phases are
    # interleaved to keep the PE dense:
    #  - both groupnorms first (ACT Sqrt all precede the first Exp); the
    #    bf16 weight copies run on ACT right after each sqrt so they block
    #    neither the groupnorm chain (DVE) nor the first QKV matmul,
    #  - batch 0 QKV with drains alternating ACT/DVE (ACT is idle there),
    #  - batch 0 scores alternating with batch 1 QKV (the PE fills
    #    exp-drain waits with b1 matmuls),
    #  - batch 1 scores alternating with batch 0 PV/transposes,
    #  - batch 0 proj while ACT finishes batch 1 exps, then batch 1 tail.
    assert BPC == 2 and NB == 8
    phase_load_gn(0)
    for b in range(BPC):
        qkv_alloc(b)
        scores_alloc(b)
        phase_pv_alloc(b)

    # batch 0 q/k chunks with drains alternating ACT/DVE; batch 1's
    # groupnorm is emitted after them so its DVE chain doesn't delay the
    # batch 0 drains (and its sqrt lands behind the ACT drains)
    u = 0
    for which in range(2):
        for m in range(KT):
            for nf in range(NF):
                qk_unit(0, which, m, nf, drain_act=(u % 2 == 0))
                u += 1
    phase_load_gn(1)

    # scores0 interleaved with b0's V production and b1's QKV (all these
    # drains are on DVE: ACT is exp'ing)
    fill_units = [("v0", nb, None, None) for nb in range(NB)]
    fill_units += [("qk", which, m, nf) for which in range(2)
                   for m in range(KT) for nf in range(NF)]
    fill_units += [("v", nb, None, None) for nb in range(NB)]
    ui = 0
    for mb in range(NB):
        scores_group(0, mb)
        for _ in range(3):
            if ui < len(fill_units):
                kind, a1, a2, a3 = fill_units[ui]
                ui += 1
                if kind == "qk":
                    qk_unit(1, a1, a2, a3)
                elif kind == "v0":
                    v_group(0, a1)
                else:
                    v_group(1, a1)
    while ui < len(fill_units):
        kind, a1, a2, a3 = fill_units[ui]
        ui += 1
        if kind == "qk":
            qk_unit(1, a1, a2, a3)
        elif kind == "v0":
            v_group(0, a1)
        else:
            v_group(1, a1)

    # scores1 interleaved with pv0 + transposes0
    for mb in range(NB):
        scores_group(1, mb)
        pv_group(0, mb)
        if mb >= 1:
            transpose_group(0, mb - 1)
    transpose_group(0, NB - 1)

    for cb in range(KT):
        for nf in range(NF):
            proj_unit(0, cb, nf)
    for nb in range(NB):
        pv_group(1, nb)
        if nb >= 1:
            transpose_group(1, nb - 1)
    transpose_group(1, NB - 1)
    for cb in range(KT):
        for nf in range(NF):
            proj_unit(1, cb, nf)


_BUILD_CACHE = {}


def _build(has_qkv_bias: bool, has_proj_bias: bool):
    key = (has_qkv_bias, has_proj_bias)
    if key in _BUILD_CACHE:
        return _BUILD_CACHE[key]
    nc = bacc.Bacc(
        "TRN2", target_bir_lowering=False, debug=False, enable_asserts=False
    )
    x_d = nc.dram_tensor("x", [BPC, C, N], BF16, kind="ExternalInput")
    gamma_d = nc.dram_tensor("gamma", [C, 1], F32, kind="ExternalInput")
    beta_d = nc.dram_tensor("beta", [C, 1], F32, kind="ExternalInput")
    wqkvT_d = nc.dram_tensor("w_qkvT", [C, 3 * C], BF16, kind="ExternalInput")
    bqkv_d = nc.dram_tensor("b_qkv", [3 * C, 1], F32, kind="ExternalInput")
    wprojT_d = nc.dram_tensor("w_projT", [C, C], BF16, kind="ExternalInput")
    bproj_d = nc.dram_tensor("b_proj", [C, 1], F32, kind="ExternalInput")
    out_d = nc.dram_tensor("out", [BPC, C, N], F32, kind="ExternalOutput")

    import ml_dtypes
    ident_np = np.eye(P, dtype=np.float32).astype(ml_dtypes.bfloat16)
    gmask_np = np.zeros((P, 4), dtype=np.float32)
    for c in range(P):
        gmask_np[c, c // GS] = 1.0 / GS
    gmask2_np = np.zeros((4, P), dtype=np.float32)
    for c in range(P):
        gmask2_np[c // GS, c] = 1.0
    ident_d = nc.inline_tensor(ident_np, "ident")
    gmask_d = nc.inline_tensor(gmask_np, "gmask")
    gmask2_d = nc.inline_tensor(gmask2_np, "gmask2")

    with tile.TileContext(nc) as tc:
        attn_tile_kernel(
            tc, out_d, x_d, gamma_d, beta_d, wqkvT_d, bqkv_d, wprojT_d,
            bproj_d, ident_d, gmask_d, gmask2_d, has_qkv_bias, has_proj_bias,
        )
    nc.compile()
    _BUILD_CACHE[key] = nc
    return nc


def kernel(**inputs) -> np.ndarray:
    x = np.ascontiguousarray(np.asarray(inputs["x"], dtype=np.float32))
    gamma = np.asarray(inputs["gamma"], np.float32).reshape(C, 1)
    beta = np.asarray(inputs["beta"], np.float32).reshape(C, 1)
    w_qkv = np.asarray(inputs["w_qkv"], np.float32)
    b_qkv = np.asarray(inputs["b_qkv"], np.float32).reshape(3 * C, 1)
    w_proj = np.asarray(inputs["w_proj"], np.float32)
    b_proj = np.asarray(inputs["b_proj"], np.float32).reshape(C, 1)

    import ml_dtypes
    wqkvT = np.ascontiguousarray(w_qkv.T).astype(ml_dtypes.bfloat16)    # [C, 3C]
    wprojT = np.ascontiguousarray(w_proj.T).astype(ml_dtypes.bfloat16)  # [C, C]
    has_qkv_bias = bool(np.any(b_qkv))
    has_proj_bias = bool(np.any(b_proj))

    nc = _build(has_qkv_bias, has_proj_bias)

    shared = {
        "gamma": np.ascontiguousarray(gamma),
        "beta": np.ascontiguousarray(beta),
        "w_qkvT": wqkvT,
        "b_qkv": np.ascontiguousarray(b_qkv),
        "w_projT": wprojT,
        "b_proj": np.ascontiguousarray(b_proj),
    }
    in_maps = []
    for core in range(NCORES):
        xm = np.ascontiguousarray(
            x[core * BPC:(core + 1) * BPC].reshape(BPC, C, N)
        ).astype(ml_dtypes.bfloat16)
        in_maps.append({"x": xm, **shared})

    res = run_bass_kernel_spmd(nc, in_maps, core_ids=list(range(NCORES)))
    out = np.concatenate(
        [r["out"].reshape(BPC, C, H, W) for r in res.results], axis=0
    )
    return np.ascontiguousarray(out.astype(np.float32))


# revision 18
# speedup vs baseline: 1.1511x; 1.1511x over previous
"""AttentionBlock Trainium2 kernel (v3: fp8 DoubleRow core + pipelined emission).

Problem: B=16, C=256, H=W=32 (N=1024 pixels), GroupNorm(8) -> 1x1-conv QKV ->
softmax attention over pixels -> 1x1-conv proj -> +residual.

Sharding: data-parallel over batch across 8 NeuronCores (2 batch elems/core),
weights replicated.  Weights are pre-transposed on the host so every matmul
operand DMAs contiguously.

Device layout per batch element (channels on partitions):
  x                  : [C=256, N=1024] fp32 (residual + groupnorm stats)
  xn                 : [C, N] bf16 (full-rate PE matmuls, fast LDWEIGHTS)
  q, k               : [C, N] fp8e4 -> scores via one DoubleRow matmul per
                       (mb, nf): contraction C=256 packed 2/partition at
                       the fp8 2x FLOP rate
  v_aug              : [N, C+8] fp8e4 (pixels on partitions; cols 256.. = ones
                       so the P@V matmul also produces the softmax row-sums)
  expS = exp(S/16-c) : [m, n] fp8e4, exp'd by ScalarE straight out of PSUM
                       (constant logit shift cancels in the normalize;
                       max logit ~5.6 -> exp in fp8e4 range, max 240)
  att^T = P@V        : 4 DoubleRow matmuls per n-block (m packed in pairs),
                       then PE-transposed to [C, n] bf16 for the proj.
Residual + proj bias are fused into the PSUM->SBUF move with one
scalar_tensor_tensor op, keeping the residual exact fp32.

Scheduling: the PE instruction stream is interleaved across phases so it
never stalls on ACT exp drains -- batch 0's score chunks alternate with
batch 1's QKV groups, and batch 1's score chunks alternate with batch 0's
PV/transpose groups.  Both activation tables (Sqrt, Exp) are preloaded by
dummy ops while the input DMAs are in flight; batch 0's QKV drains run on
the otherwise-idle ACT engine, batch 1's on DVE.
"""

from contextlib import ExitStack

import numpy as np

import concourse.bass as bass
import concourse.tile as tile
from concourse import bacc, mybir
from concourse._compat import with_exitstack
from concourse.bass_utils import run_bass_kernel_spmd

# Problem constants (hardcoded per contract)
B, C, H, W = 16, 256, 32, 32
N = H * W            # 1024 pixels
G = 8                # groups
GS = C // G          # 32 channels / group
NCORES = 8
BPC = B // NCORES    # batch elems per core
EPS = 1e-5
P = 128              # partitions
KT = C // P          # 2 c-tiles
NB = N // P          # 8 pixel blocks of 128
NF = N // 512        # 2 free chunks of 512
EXP_SHIFT = -1.5     # constant logit shift; cancels in softmax normalize
F32 = mybir.dt.float32
BF16 = mybir.dt.bfloat16
FP8 = mybir.dt.float8e4
AF = mybir.ActivationFunctionType
OP = mybir.AluOpType
DR = mybir.MatmulPerfMode.DoubleRow


@with_exitstack
def attn_tile_kernel(
    ctx: ExitStack,
    tc: tile.TileContext,
    out_d,
    x_d,
    gamma_d,
    beta_d,
    wqkvT_d,
    bqkv_d,
    wprojT_d,
    bproj_d,
    ident_d,
    gmask_d,
    gmask2_d,
    has_qkv_bias: bool,
    has_proj_bias: bool,
):
    nc = tc.nc

    consts = ctx.enter_context(tc.tile_pool(name="consts", bufs=1))
    gn = ctx.enter_context(tc.tile_pool(name="gn", bufs=2))
    big = ctx.enter_context(tc.tile_pool(name="big", bufs=2))
    es = ctx.enter_context(tc.tile_pool(name="es", bufs=2))
    # PSUM budget (8 banks), split by draining engine so phases don't
    # block each other on slots:
    #   mm  = 5x [128,512] qkv/scores/proj psums (DVE/ACT drains) (5 banks)
    #   pv  = 3x [128,264] v/PV accum + transposes + gn stats     (3 banks)
    psmm = ctx.enter_context(tc.tile_pool(name="psmm", bufs=5, space="PSUM"))
    psqp = psmm
    pspv = ctx.enter_context(tc.tile_pool(name="pspv", bufs=3, space="PSUM"))
    pssm = pspv

    state = {}

    # batch inputs go first: they gate the groupnorm -> QKV critical path,
    # while weights aren't needed until the first matmul ~8us later.
    # x0 in fine chunks (bn_stats starts per chunk), x1 whole-tile on the
    # gpsimd queue (latency uncritical, keeps the sync queue short).
    xt0 = big.tile([P, KT, N], BF16, tag="xt")
    for kt in range(KT):
        for sub in range(2):
            nc.sync.dma_start(
                xt0[:, kt, sub * 512:(sub + 1) * 512],
                x_d[0, kt * P:(kt + 1) * P, sub * 512:(sub + 1) * 512])
    state[0] = {"xt": xt0}
    xt1 = big.tile([P, KT, N], BF16, tag="xt")
    for kt in range(KT):
        nc.sync.dma_start(xt1[:, kt, :], x_d[1, kt * P:(kt + 1) * P, :])
    state[1] = {"xt": xt1}

    # ---- constants / weights (loaded once, replicated across cores) ----
    # DMA issue cost is ~0.7-0.9us per descriptor per engine queue, so the
    # issues are spread across otherwise-idle engine queues: x0 owns the
    # sync queue (it gates groupnorm), x1 + stores go via gpsimd, small
    # groupnorm consts via the tensor queue, weights via the scalar queue.
    gmask_t = consts.tile([P, 4], F32)
    nc.gpsimd.dma_start(gmask_t[:], gmask_d.ap())
    gmask2_t = consts.tile([4, P], F32)
    nc.gpsimd.dma_start(gmask2_t[:], gmask2_d.ap())
    gamma_t = consts.tile([P, KT], F32)
    beta_t = consts.tile([P, KT], F32)
    for kt in range(KT):
        nc.gpsimd.dma_start(gamma_t[:, kt:kt + 1], gamma_d[kt * P:(kt + 1) * P, :])
        nc.gpsimd.dma_start(beta_t[:, kt:kt + 1], beta_d[kt * P:(kt + 1) * P, :])
    eps_t = consts.tile([P, 1], F32)
    nc.vector.memset(eps_t, EPS)
    # warm-up matmuls on junk data: the PE clock ramps to full speed only
    # after ~3us of continuous work, so spin it up while DMAs are in flight
    warm_sb = consts.tile([P, 512], BF16)
    nc.vector.memset(warm_sb[:], 0.0)
    for _ in range(16):
        warm_ps = psmm.tile([P, 512], F32, tag="mm", name="warm_ps")
        nc.tensor.matmul(warm_ps[:], warm_sb[:, 0:P], warm_sb[:],
                         start=True, stop=True)
    shift_t = consts.tile([P, 1], F32)
    nc.vector.memset(shift_t, EXP_SHIFT)
    # preload both ACT tables (Sqrt for groupnorm, Exp for softmax) while
    # the input DMAs are still in flight so neither load sits on the
    # critical path later
    warm_t = consts.tile([P, 1], F32)
    nc.scalar.activation(warm_t[:], eps_t[:], AF.Sqrt, bias=eps_t[:], scale=1.0)
    nc.scalar.activation(warm_t[:], eps_t[:], AF.Exp, bias=shift_t[:], scale=1.0)
    # weights arrive pre-cast to bf16 from the host and go on the sync
    # queue AFTER x0/x1: DMA transfers share bandwidth, so every byte saved
    # or deferred here pulls the groupnorm -> QKV critical path earlier
    wqkv_t = consts.tile([P, KT, 3 * C], BF16)
    for kt in range(KT):
        nc.sync.dma_start(wqkv_t[:, kt, :], wqkvT_d[kt * P:(kt + 1) * P, :])
    wproj_b = consts.tile([P, KT, C], BF16)
    for kt in range(KT):
        nc.sync.dma_start(wproj_b[:, kt, :], wprojT_d[kt * P:(kt + 1) * P, :])
    ident_t = consts.tile([P, P], BF16)
    nc.sync.dma_start(ident_t[:], ident_d.ap())
    # q/k biases as per-partition columns (q: rows 0..255, k: 256..511)
    if has_qkv_bias:
        bqk_t = consts.tile([P, 2 * KT], F32)
        for j in range(2 * KT):
            nc.gpsimd.dma_start(bqk_t[:, j:j + 1], bqkv_d[j * P:(j + 1) * P, :])
        # v bias lives on the free axis of v_aug -> needs a row layout + ones col
        bv_row = consts.tile([1, C], F32)
        nc.gpsimd.dma_start(
            bv_row[:], bqkv_d[2 * C:3 * C, :].rearrange("c one -> one c"))
        ones_row_f32 = consts.tile([1, P], F32)
        nc.vector.memset(ones_row_f32, 1.0)
    if has_proj_bias:
        bproj_t = consts.tile([P, KT], F32)
        for kt in range(KT):
            nc.gpsimd.dma_start(bproj_t[:, kt:kt + 1], bproj_d[kt * P:(kt + 1) * P, :])

    def phase_load_gn(b):
        # ---- group norm stats -> per-channel scale/bias -> xn (bf16) ----
        # both c-tiles flow through one PE reduce + one [P,2]-wide chain
        st = state[b]
        xt = st["xt"]
        stats = gn.tile([P, KT, 2, 6], F32, tag="stats")
        mv = gn.tile([P, KT, 2], F32, tag="mv")
        gst = gn.tile([P, 2, KT], F32, tag="gst")  # stat-major: means | m2s
        for kt in range(KT):
            for sub in range(2):
                nc.vector.bn_stats(stats[:, kt, sub, :], xt[:, kt, sub * 512:(sub + 1) * 512])
            nc.vector.bn_aggr(mv[:, kt, :], stats[:, kt, :, :])
            nc.vector.tensor_copy(gst[:, 0, kt:kt + 1], mv[:, kt, 0:1])
            nc.vector.tensor_scalar(
                out=gst[:, 1, kt:kt + 1], in0=mv[:, kt, 0:1],
                scalar1=mv[:, kt, 0:1], scalar2=mv[:, kt, 1:2],
                op0=OP.mult, op1=OP.add,
            )
        gsum = pssm.tile([4, 2 * KT], F32, tag="pv")
        nc.tensor.matmul(gsum[:], gmask_t[:], gst[:, :, :], start=True, stop=True)
        gsb = gn.tile([4, 2 * KT], F32, tag="gsb")
        nc.vector.tensor_copy(gsb[:], gsum[:])
        gbc = pssm.tile([P, 2 * KT], F32, tag="pv")
        nc.tensor.matmul(gbc[:], gmask2_t[:], gsb[:], start=True, stop=True)
        gch = gn.tile([P, 2, KT], F32, tag="gch")  # means | m2s per channel
        nc.vector.tensor_copy(gch[:], gbc[:])
        m2 = gn.tile([P, KT], F32, tag="m2")
        nc.vector.tensor_mul(m2[:], gch[:, 0, :], gch[:, 0, :])
        varg = gn.tile([P, KT], F32, tag="varg")
        nc.vector.tensor_sub(varg[:], gch[:, 1, :], m2[:])
        sd = gn.tile([P, KT], F32, tag="sd")
        nc.scalar.activation(sd[:], varg[:], AF.Sqrt, bias=eps_t[:], scale=1.0)
        rstd = gn.tile([P, KT], F32, tag="rstd")
        nc.vector.reciprocal(rstd[:], sd[:])
        scale_c = gn.tile([P, KT], F32, tag="scale_c")
        nc.vector.tensor_mul(scale_c[:], rstd[:], gamma_t[:])
        tmp = gn.tile([P, KT], F32, tag="tmp")
        nc.vector.tensor_mul(tmp[:], gch[:, 0, :], scale_c[:])
        bias_c = gn.tile([P, KT], F32, tag="bias_c")
        nc.vector.tensor_sub(bias_c[:], beta_t[:], tmp[:])
        xn = big.tile([P, KT, N], BF16, tag="xn")
        for kt in range(KT):
            nc.vector.tensor_scalar(
                out=xn[:, kt, :], in0=xt[:, kt, :],
                scalar1=scale_c[:, kt:kt + 1], scalar2=bias_c[:, kt:kt + 1],
                op0=OP.mult, op1=OP.add,
            )
        st["xn"] = xn

    def qkv_alloc(b):
        st = state[b]
        st["q"] = big.tile([P, KT, N], FP8, tag=f"q{b}", name=f"q{b}")
        st["k"] = big.tile([P, KT, N], FP8, tag=f"k{b}", name=f"k{b}")
        st["v"] = big.tile([P, NB, C + 8], FP8, tag=f"v{b}", name=f"v{b}")

    def qk_unit(b, which, m, nf, drain_act=False):
        # one [128, 512] chunk of q (which=0) or k (which=1); single psum,
        # kt accumulated in place so the chunk drains as soon as possible
        st = state[b]
        xn = st["xn"]
        dst = st["q"] if which == 0 else st["k"]
        ocol = which * C + m * P
        ps = psqp.tile([P, 512], F32, tag="mm")
        for kt in range(KT):
            nc.tensor.matmul(
                ps[:],
                wqkv_t[:, kt, ocol:ocol + P],
                xn[:, kt, nf * 512:(nf + 1) * 512],
                start=(kt == 0), stop=(kt == KT - 1),
            )
        dstap = dst[:, m, nf * 512:(nf + 1) * 512]
        if has_qkv_bias:
            nc.vector.tensor_scalar_add(
                dstap, ps[:], bqk_t[:, 2 * which + m:2 * which + m + 1])
        elif drain_act:
            nc.scalar.copy(dstap, ps[:])
        else:
            nc.vector.tensor_copy(dstap, ps[:])

    def v_group(b, nb):
        # one pixel-block of v^T (pixels on partitions) + ones columns
        st = state[b]
        xn, v_aug = st["xn"], st["v"]
        ps = pspv.tile([P, C + 8], F32, tag="pv")
        for kt in range(KT):
            nc.tensor.matmul(
                ps[:, 0:C],
                xn[:, kt, nb * P:(nb + 1) * P],
                wqkv_t[:, kt, 2 * C:3 * C],
                start=(kt == 0), stop=(kt == KT - 1 and not has_qkv_bias),
            )
        if has_qkv_bias:
            nc.tensor.matmul(ps[:, 0:C], ones_row_f32[:], bv_row[:],
                             start=False, stop=True)
        nc.vector.tensor_copy(v_aug[:, nb, 0:C], ps[:, 0:C])
        nc.vector.memset(v_aug[:, nb, C:C + 8], 1.0)

    def scores_alloc(b):
        state[b]["expS"] = es.tile([P, NB, N], FP8, tag=f"expS{b}",
                                   name=f"expS{b}")

    def scores_group(b, mb):
        # S^T = k^T q for one m-block: a single DoubleRow matmul per nf
        # (both c-tiles contracted in one pass), exp'd chunkwise by ACT
        st = state[b]
        q_sb, k_sb, expS = st["q"], st["k"], st["expS"]
        for nf in range(NF):
            ps = psmm.tile([P, 512], F32, tag="mm")
            nc.tensor.matmul(
                ps[:],
                k_sb[:, :, mb * P:(mb + 1) * P],
                q_sb[:, :, nf * 512:(nf + 1) * 512],
                start=True, stop=True, perf_mode=DR,
            )
            nc.scalar.activation(expS[:, mb, nf * 512:(nf + 1) * 512], ps[:],
                                 AF.Exp, bias=shift_t[:], scale=1.0 / 16.0)

    def pv_group(b, nb):
        # one n-block of att^T = softmax(S) @ V (col C holds row-sums);
        # m contracted in 4 DoubleRow pairs
        st = state[b]
        expS, v_aug, attT = st["expS"], st["v"], st["attT"]
        ps = pspv.tile([P, C + 8], F32, tag="pv")
        for j in range(NB // 2):
            nc.tensor.matmul(
                ps[:, 0:C + 8],
                expS[:, 2 * j:2 * j + 2, nb * P:(nb + 1) * P],
                v_aug[:, 2 * j:2 * j + 2, 0:C + 8],
                start=(j == 0), stop=(j == NB // 2 - 1), perf_mode=DR,
            )
        rinv = gn.tile([P, 1], F32, tag="rinv")
        nc.vector.reciprocal(rinv[:], ps[:, C:C + 1])
        if b == 0:
            # batch 0 normalizes on DVE (ACT may still be draining exps)
            nc.vector.tensor_scalar_mul(attT[:, nb, :], ps[:, 0:C], rinv[:])
        else:
            # batch 1 normalizes on ACT's free affine (idle post-softmax):
            # out = Copy(in * scale), scale = per-partition rinv
            nc.scalar.activation(attT[:, nb, :], ps[:, 0:C], AF.Copy,
                                 bias=0.0, scale=rinv[:])

    def phase_pv_alloc(b):
        st = state[b]
        st["attT"] = big.tile([P, NB, C], BF16, tag=f"attT{b}", name=f"attT{b}")
        st["att"] = big.tile([P, KT, N], BF16, tag=f"att{b}", name=f"att{b}")
        st["out_sb"] = big.tile([P, KT, N], F32, tag=f"outsb{b}",
                                name=f"outsb{b}")

    def transpose_group(b, nb):
        # transpose one att^T block -> att [*, n-block] via PE (both c-blocks)
        st = state[b]
        attT, att = st["attT"], st["att"]
        for cb in range(KT):
            pt = pssm.tile([P, P], BF16, tag="pv")
            nc.tensor.transpose(pt[:], attT[:, nb, cb * P:(cb + 1) * P], ident_t[:])
            nc.vector.tensor_copy(att[:, cb, nb * P:(nb + 1) * P], pt[:])

    def proj_unit(b, cb, nf):
        # one [128, 512] proj chunk + fused (+bias)+residual on the
        # PSUM->SBUF move, then store (DMA issues spread across idle queues)
        st = state[b]
        att, xt, out_sb = st["att"], st["xt"], st["out_sb"]
        ps = psqp.tile([P, 512], F32, tag="mm")
        for kt in range(KT):
            nc.tensor.matmul(
                ps[:],
                wproj_b[:, kt, cb * P:(cb + 1) * P],
                att[:, kt, nf * 512:(nf + 1) * 512],
                start=(kt == 0), stop=(kt == KT - 1),
            )
        bias_arg = bproj_t[:, cb:cb + 1] if has_proj_bias else 0.0
        nc.vector.scalar_tensor_tensor(
            out=out_sb[:, cb, nf * 512:(nf + 1) * 512],
            in0=ps[:], scalar=bias_arg,
            in1=xt[:, cb, nf * 512:(nf + 1) * 512],
            op0=OP.add, op1=OP.add,
        )
        dma_eng = (nc.gpsimd if b == 0 else (nc.sync if nf == 0 else nc.scalar))
        dma_eng.dma_start(
            out_d[b, cb * P:(cb + 1) * P, nf * 512:(nf + 1) * 512],
            out_sb[:, cb, nf * 512:(nf + 1) * 512])

    # ---- software-pipelined emission ------------------------------------
    # Engines run their per-engine streams in order, so phases are
    # interleaved to keep the PE dense:
    #  - both groupnorms first (ACT Sqrt all precede the first Exp); the
    #    bf16 weight copies run on ACT right after each sqrt so they block
    #    neither the groupnorm chain (DVE) nor the first QKV matmul,
    #  - batch 0 QKV with drains alternating ACT/DVE (ACT is idle there),
    #  - batch 0 scores alternating with batch 1 QKV (the PE fills
    #    exp-drain waits with b1 matmuls),
    #  - batch 1 scores alternating with batch 0 PV/transposes,
    #  - batch 0 proj while ACT finishes batch 1 exps, then batch 1 tail.
    assert BPC == 2 and NB == 8
    phase_load_gn(0)
    for b in range(BPC):
        qkv_alloc(b)
        scores_alloc(b)
        phase_pv_alloc(b)

    # batch 0 q/k chunks with drains alternating ACT/DVE; batch 1's
    # groupnorm is emitted after them so its DVE chain doesn't delay the
    # batch 0 drains (and its sqrt lands behind the ACT drains)
    u = 0
    for which in range(2):
        for m in range(KT):
            for nf in range(NF):
                qk_unit(0, which, m, nf, drain_act=(u % 2 == 0))
                u += 1
    phase_load_gn(1)
    for nb in range(NB):
        v_group(0, nb)

    # scores0 interleaved with qkv1 (all b1 drains on DVE: ACT is exp'ing)
    fill_units = [("qk", which, m, nf) for which in range(2)
                  for m in range(KT) for nf in range(NF)]
    fill_units += [("v", nb, None, None) for nb in range(NB)]
    ui = 0
    for mb in range(NB):
        scores_group(0, mb)
        for _ in range(2):
            if ui < len(fill_units):
                kind, a1, a2, a3 = fill_units[ui]
                ui += 1
                if kind == "qk":
                    qk_unit(1, a1, a2, a3)
                else:
                    v_group(1, a1)
    while ui < len(fill_units):
        kind, a1, a2, a3 = fill_units[ui]
        ui += 1
        if kind == "qk":
            qk_unit(1, a1, a2, a3)
        else:
            v_group(1, a1)

    # scores1 interleaved with pv0 + transposes0
    for mb in range(NB):
        scores_group(1, mb)
        pv_group(0, mb)
        if mb >= 1:
            transpose_group(0, mb - 1)
    transpose_group(0, NB - 1)

    for cb in range(KT):
        for nf in range(NF):
            proj_unit(0, cb, nf)
    for nb in range(NB):
        pv_group(1, nb)
        if nb >= 1:
            transpose_group(1, nb - 1)
    transpose_group(1, NB - 1)
    for cb in range(KT):
        for nf in range(NF):
            proj_unit(1, cb, nf)


_BUILD_CACHE = {}


def _build(has_qkv_bias: bool, has_proj_bias: bool):
    key = (has_qkv_bias, has_proj_bias)
    if key in _BUILD_CACHE:
        return _BUILD_CACHE[key]
    nc = bacc.Bacc(
        "TRN2", target_bir_lowering=False, debug=False, enable_asserts=False
    )
    x_d = nc.dram_tensor("x", [BPC, C, N], BF16, kind="ExternalInput")
    gamma_d = nc.dram_tensor("gamma", [C, 1], F32, kind="ExternalInput")
    beta_d = nc.dram_tensor("beta", [C, 1], F32, kind="ExternalInput")
    wqkvT_d = nc.dram_tensor("w_qkvT", [C, 3 * C], BF16, kind="ExternalInput")
    bqkv_d = nc.dram_tensor("b_qkv", [3 * C, 1], F32, kind="ExternalInput")
    wprojT_d = nc.dram_tensor("w_projT", [C, C], BF16, kind="ExternalInput")
    bproj_d = nc.dram_tensor("b_proj", [C, 1], F32, kind="ExternalInput")
    out_d = nc.dram_tensor("out", [BPC, C, N], F32, kind="ExternalOutput")

    import ml_dtypes
    ident_np = np.eye(P, dtype=np.float32).astype(ml_dtypes.bfloat16)
    gmask_np = np.zeros((P, 4), dtype=np.float32)
    for c in range(P):
        gmask_np[c, c // GS] = 1.0 / GS
    gmask2_np = np.zeros((4, P), dtype=np.float32)
    for c in range(P):
        gmask2_np[c // GS, c] = 1.0
    ident_d = nc.inline_tensor(ident_np, "ident")
    gmask_d = nc.inline_tensor(gmask_np, "gmask")
    gmask2_d = nc.inline_tensor(gmask2_np, "gmask2")

    with tile.TileContext(nc) as tc:
        attn_tile_kernel(
            tc, out_d, x_d, gamma_d, beta_d, wqkvT_d, bqkv_d, wprojT_d,
            bproj_d, ident_d, gmask_d, gmask2_d, has_qkv_bias, has_proj_bias,
        )
    nc.compile()
    _BUILD_CACHE[key] = nc
    return nc


def kernel(**inputs) -> np.ndarray:
    x = np.ascontiguousarray(np.asarray(inputs["x"], dtype=np.float32))
    gamma = np.asarray(inputs["gamma"], np.float32).reshape(C, 1)
    beta = np.asarray(inputs["beta"], np.float32).reshape(C, 1)
    w_qkv = np.asarray(inputs["w_qkv"], np.float32)
    b_qkv = np.asarray(inputs["b_qkv"], np.float32).reshape(3 * C, 1)
    w_proj = np.asarray(inputs["w_proj"], np.float32)
    b_proj = np.asarray(inputs["b_proj"], np.float32).reshape(C, 1)

    import ml_dtypes
    wqkvT = np.ascontiguousarray(w_qkv.T).astype(ml_dtypes.bfloat16)    # [C, 3C]
    wprojT = np.ascontiguousarray(w_proj.T).astype(ml_dtypes.bfloat16)  # [C, C]
    has_qkv_bias = bool(np.any(b_qkv))
    has_proj_bias = bool(np.any(b_proj))

    nc = _build(has_qkv_bias, has_proj_bias)

    shared = {
        "gamma": np.ascontiguousarray(gamma),
        "beta": np.ascontiguousarray(beta),
        "w_qkvT": wqkvT,
        "b_qkv": np.ascontiguousarray(b_qkv),
        "w_projT": wprojT,
        "b_proj": np.ascontiguousarray(b_proj),
    }
    in_maps = []
    for core in range(NCORES):
        xm = np.ascontiguousarray(
            x[core * BPC:(core + 1) * BPC].reshape(BPC, C, N)
        ).astype(ml_dtypes.bfloat16)
        in_maps.append({"x": xm, **shared})

    res = run_bass_kernel_spmd(nc, in_maps, core_ids=list(range(NCORES)))
    out = np.concatenate(
        [r["out"].reshape(BPC, C, H, W) for r in res.results], axis=0
    )
    return np.ascontiguousarray(out.astype(np.float32))
